# revision 21
# baseline (speedup 1.0000x reference)
"""Causal self-attention (GPT-style, B=2 T=2048 C=1024 H=16) on 8 Trainium2 cores.

Sharding (Megatron-style): data-parallel over batch (cores 0-3 own b=0,
cores 4-7 own b=1) x tensor-parallel over heads (4 heads/core, c_attn
column-split / c_proj row-split). Each core emits a partial [T, C] output;
the host unshard step sums the 4 partials per batch (the TP all-reduce) and
stacks the batches.

Per-core device program, software-pipelined per 512-query window:
  window ts: QKV projection of token slice ts (k-outer over the 8 C-subtiles
  so the cold-start DMA stream paces the matmuls), then causal flash-style
  attention for query slice qs=ts (its keys/values tiles 0..4ts+3 are all
  resident by then), with the previous slice's output projection and
  normalization lagged into the stream so their latency hides under matmuls.

Dtypes: x/W_attn tiles and Q/K/V/P in bf16 (PE runs bf16 at 1 cycle/row at
any width, enabling diagonal-tile narrowing); all PSUM accumulation fp32;
O, W_proj and the output path fp32/f32r.
"""

import numpy as np

import concourse.bass as bass  # noqa: F401  (re-exported types)
import concourse.mybir as mybir
import concourse.tile as tile
from concourse import bacc
from concourse.bass_utils import run_bass_kernel_spmd

B, T, C = 2, 2048, 1024
H, DH = 16, 64
NCORES = 8
GROUP = 4            # cores per batch (tensor-parallel group)
HPC = H // GROUP     # heads per core
P = 128
KO = C // P          # k-subtiles in the C contraction
TQ = 512             # query-slice width (PSUM bank)
NTS = T // TQ
NTK = T // P

F32 = mybir.dt.float32
F32R = mybir.dt.float32r
BF16 = mybir.dt.bfloat16

_CACHE: dict = {}
DISABLE_NARROW = False
DEBUG_DUMPS = False


def _build_nc(n_iters=1):
    nc = bacc.Bacc("TRN2", target_bir_lowering=False, debug=False, num_devices=NCORES)
    xT = nc.dram_tensor("xT", [C, T], BF16, kind="ExternalInput")
    wqk = nc.dram_tensor("wqk", [C, 512], BF16, kind="ExternalInput")
    wv = nc.dram_tensor("wv", [C, 256], BF16, kind="ExternalInput")
    wp = nc.dram_tensor("wp", [2 * P, C], F32R, kind="ExternalInput")
    bqk = nc.dram_tensor("bqk", [512], F32, kind="ExternalInput")
    bv = nc.dram_tensor("bv", [256], F32, kind="ExternalInput")
    bp = nc.dram_tensor("bp", [C], F32, kind="ExternalInput")
    mask = nc.dram_tensor("mask", [P, 132], BF16, kind="ExternalInput")
    y = nc.dram_tensor("y", [T, C], F32, kind="ExternalOutput")
    dbg = {}
    if DEBUG_DUMPS:
        dbg["qt"] = nc.dram_tensor("qt_dbg", [P, 2, T], F32, kind="ExternalOutput")
        dbg["kt"] = nc.dram_tensor("kt_dbg", [P, 2, T], F32, kind="ExternalOutput")
        dbg["vt"] = nc.dram_tensor("vt_dbg", [P, NTK, HPC, DH + 1], F32, kind="ExternalOutput")
        dbg["ot"] = nc.dram_tensor("ot_dbg", [P, 2, T], F32, kind="ExternalOutput")

    with tile.TileContext(nc) as tc:
        if n_iters == 1:
            _emit(tc, xT, wqk, wv, wp, bqk, bv, bp, mask, y, dbg)
        else:
            with tc.For_i(0, n_iters, 1):
                _emit(tc, xT, wqk, wv, wp, bqk, bv, bp, mask, y, dbg)
    nc.compile()
    return nc


def _emit(tc, xT, wqk, wv, wp, bqk, bv, bp, mask, y, dbg={}):
    nc = tc.nc
    Exp = mybir.ActivationFunctionType.Exp
    Ident = mybir.ActivationFunctionType.Identity
    mm = nc.tensor.matmul

    with (
        tc.tile_pool(name="consts", bufs=1) as consts,
        tc.tile_pool(name="xp", bufs=2) as xp,
        tc.tile_pool(name="ptp", bufs=5) as ptp,
        tc.tile_pool(name="smp", bufs=3) as smp,
        tc.tile_pool(name="obp", bufs=3) as obp,
        tc.tile_pool(name="psp", bufs=2, space="PSUM") as psp,
        tc.tile_pool(name="psop", bufs=2, space="PSUM") as psop,
    ):
        # -------- startup DMAs: first-needed first, interleaved per k ------
        wqk_r = wqk.ap().rearrange("(ko p) m -> p ko m", p=P)
        xTr = xT.ap().rearrange("(ko p) t -> p ko t", p=P)
        wqk_sb = consts.tile([P, KO, 512], BF16, tag="wqk")
        xt0 = xp.tile([P, KO, TQ], BF16, tag="xt")
        wv_sb = consts.tile([P, KO, 256], BF16, tag="wv")
        wv_r = wv.ap().rearrange("(ko p) m -> p ko m", p=P)
        warm = consts.tile([P, 1], F32, tag="warm")
        nc.vector.memset(warm[:], 0.0)
        nc.scalar.activation(warm[:], warm[:],
                             mybir.ActivationFunctionType.Exp)
        bqk_sb = consts.tile([P, 4], F32, tag="bqk")
        for k in range(KO):
            nc.sync.dma_start(wqk_sb[:, k], wqk_r[:, k])
            nc.sync.dma_start(xt0[:, k], xTr[:, k, 0:TQ])
            if k == 0:
                nc.sync.dma_start(bqk_sb[:],
                                  bqk.ap().rearrange("(m p) -> p m", p=P))
        nc.sync.dma_start(wv_sb[:], wv_r)
        bv_sb = consts.tile([P, 256], F32, tag="bv")
        nc.sync.dma_start(bv_sb[:], bv.ap().partition_broadcast(P))
        mask_sb = consts.tile([P, 132], BF16, tag="mask")
        nc.sync.dma_start(mask_sb[:], mask[:])

        QT = consts.tile([P, 2, T], BF16, tag="QT")   # q^T; head 2g on parts 0-63
        KTt = consts.tile([P, 2, T], BF16, tag="KT")  # k^T, same layout
        Vt = consts.tile([P, NTK, HPC, DH + 1], BF16, tag="V")  # v + ones col
        Ot = consts.tile([P, 2, T], F32R, tag="O")    # unnorm-then-norm O^T

        # ones column for the softmax-denominator trick (mask col 128 is ones)
        nc.vector.tensor_copy(
            Vt[:, :, :, DH:DH + 1],
            mask_sb[:, P:P + 1].rearrange("p (a b c) -> p a b c", a=1, b=1)
            .to_broadcast((P, NTK, HPC, 1)),
        )

        wp_sb = consts.tile([P, 2, C], F32R, tag="wp")
        bp_sb = consts.tile([P, C], F32, tag="bp")

        def emit_qkv(ts, xt):
            sl = slice(ts * TQ, (ts + 1) * TQ)
            # Q^T and K^T, k-outer: 4 live accumulations in 2 psum tiles so
            # each arriving k-subtile unlocks 4 matmuls (cold-start pacing).
            pa = psp.tile([P, 2 * TQ], F32, tag="s")
            pb = psp.tile([P, 2 * TQ], F32, tag="s")
            for k in range(KO):
                f, l = (k == 0), (k == KO - 1)
                mm(pa[:, 0:TQ], wqk_sb[:, k, 0:P], xt[:, k], start=f, stop=l)
                mm(pa[:, TQ:2 * TQ], wqk_sb[:, k, P:2 * P], xt[:, k], start=f, stop=l)
                mm(pb[:, 0:TQ], wqk_sb[:, k, 2 * P:3 * P], xt[:, k], start=f, stop=l)
                mm(pb[:, TQ:2 * TQ], wqk_sb[:, k, 3 * P:4 * P], xt[:, k], start=f, stop=l)
            for g in range(2):
                nc.scalar.activation(QT[:, g, sl], pa[:, g * TQ:(g + 1) * TQ],
                                     Ident, bias=bqk_sb[:, g:g + 1])
            for g in range(2):
                nc.scalar.activation(KTt[:, g, sl], pb[:, g * TQ:(g + 1) * TQ],
                                     Ident, bias=bqk_sb[:, 2 + g:3 + g])
            # V natural, 4 token chunks packed in one psum tile
            pv = psp.tile([P, 2 * TQ], F32, tag="v", bufs=1)
            for j in range(4):
                for k in range(KO):
                    mm(pv[:, j * 256:(j + 1) * 256], xt[:, k, j * P:(j + 1) * P],
                       wv_sb[:, k], start=(k == 0), stop=(k == KO - 1))
            for j in range(4):
                nc.vector.tensor_add(
                    Vt[:, 4 * ts + j, :, 0:DH],
                    pv[:, j * 256:(j + 1) * 256].rearrange(
                        "p (h d) -> p h d", h=HPC),
                    bv_sb[:].rearrange("p (h d) -> p h d", h=HPC),
                )

        def emit_proj(t, eng):
            # y[t*128:(t+1)*128, :] = O^T[:, t-chunk].T @ Wp + bp
            pp = psp.tile([P, 2 * TQ], F32, tag="s")
            for ns in range(C // TQ):
                for g in range(2):
                    mm(pp[:, ns * TQ:(ns + 1) * TQ],
                       Ot[:, g, t * P:(t + 1) * P],
                       wp_sb[:, g, ns * TQ:(ns + 1) * TQ],
                       start=(g == 0), stop=(g == 1))
            ob = obp.tile([P, C], F32, tag="ob")
            eng.tensor_add(ob[:], pp[:], bp_sb[:])
            nc.sync.dma_start(y[t * P:(t + 1) * P, :], ob[:])

        def emit_norm(job):
            # rows 0..63 of po = unnormalized O^T, row 64 = denominator
            po, g, pb_, qsl = job
            den = smp.tile([1, TQ], F32, tag="den")
            nc.scalar.activation(den[:], po[DH:DH + 1, :],
                                 mybir.ActivationFunctionType.Identity)
            rec1 = smp.tile([1, TQ], F32, tag="rec1")
            nc.vector.reciprocal_approx_fast(rec1[:], den[:])
            recb = smp.tile([DH, TQ], F32, tag="recb")
            nc.gpsimd.partition_broadcast(recb[:], rec1[:])
            nc.vector.tensor_mul(Ot[pb_:pb_ + DH, g, qsl], po[0:DH, :], recb[:])

        LA = 3  # lookahead in tk tiles between S^T/exp production and P^T@V
        pending_norm = []
        half_proj = []

        def emit_attn(qs):
            qsl = slice(qs * TQ, (qs + 1) * TQ)
            ntk = 4 * qs + 4  # causal: tk tiles 0 .. 4qs+3
            for hp in range(2):
                ha, hb = 2 * hp, 2 * hp + 1
                po_a = psop.tile([DH + 1, TQ], F32, tag="pv")
                po_b = psop.tile([DH + 1, TQ], F32, tag="pv")
                pts = []

                def emit_pv(tk, po_a=po_a, po_b=po_b, pts=pts, ntk=ntk,
                            ha=ha, hb=hb, qs=qs):
                    pt2, off = pts[tk]
                    f, l = (tk == 0), (tk == ntk - 1)
                    mm(po_a[:, off:TQ], Vt[:, tk, ha, :], pt2[:, off:TQ],
                       start=f, stop=l, skip_group_check=True)
                    mm(po_b[:, off:TQ], Vt[:, tk, hb, :], pt2[:, TQ + off:2 * TQ],
                       start=f, stop=l, skip_group_check=True)

                for tk in range(ntk):
                    j = tk - 4 * qs   # >= 0 -> diagonal-band tile
                    off = P * max(j, 0)
                    if DISABLE_NARROW:
                        off = 0
                    # S^T both heads: row-tiled matmuls, valid columns only
                    pss = psp.tile([P, 2 * TQ], F32, tag="s")
                    mm(pss[:, off:TQ],
                       KTt[0:DH, hp, tk * P:(tk + 1) * P],
                       QT[0:DH, hp, qs * TQ + off:(qs + 1) * TQ],
                       start=True, stop=True)
                    mm(pss[:, TQ + off:2 * TQ],
                       KTt[DH:P, hp, tk * P:(tk + 1) * P],
                       QT[DH:P, hp, qs * TQ + off:(qs + 1) * TQ],
                       start=True, stop=True)
                    pt2 = ptp.tile([P, 2 * TQ], BF16, tag="pt")
                    nc.scalar.activation(
                        pt2[:].rearrange("p (h w) -> p h w", h=2)[:, :, off:TQ],
                        pss[:].rearrange("p (h w) -> p h w", h=2)[:, :, off:TQ],
                        Exp, scale=0.125)
                    if j >= 0:  # mask only the 128-wide triangle block
                        moff = P * j
                        for hh in range(2):
                            tri = pt2[:, hh * TQ + moff:hh * TQ + moff + P]
                            nc.vector.tensor_mul(tri, tri, mask_sb[:, 0:P])
                    pts.append((pt2[:], off))
                    if tk == 0:
                        while pending_norm:
                            emit_norm(pending_norm.pop(0))
                    if hp == 1 and qs == NTS - 1 and tk == 1:
                        t = 4 * qs
                        pp = psp.tile([P, 2 * TQ], F32, tag="v", bufs=1)
                        for ns in range(C // TQ):
                            mm(pp[:, ns * TQ:(ns + 1) * TQ],
                               Ot[:, 0, t * P:(t + 1) * P],
                               wp_sb[:, 0, ns * TQ:(ns + 1) * TQ],
                               start=True, stop=False,
                               skip_group_check=True)
                        half_proj.append((t, pp))
                    if hp == 0 and qs >= 1 and tk in (1, 2, 3, 4):
                        emit_proj(4 * (qs - 1) + tk - 1, nc.vector)
                    if tk >= LA:
                        emit_pv(tk - LA)
                for tk in range(max(0, ntk - LA), ntk):
                    emit_pv(tk)
                pending_norm.append((po_a, hp, 0, qsl))
                pending_norm.append((po_b, hp, DH, qsl))

        # ------------------- software-pipelined windows -------------------
        for ts in range(NTS):
            xt = xt0 if ts == 0 else xt_next  # noqa: F821
            emit_qkv(ts, xt)
            if ts == 0:
                # late consts + first prefetch enter the DMA queue here
                nc.sync.dma_start(wp_sb[:], wp.ap().rearrange("(g p) c -> p g c", p=P))
                nc.sync.dma_start(bp_sb[:], bp.ap().partition_broadcast(P))
            if ts + 1 < NTS:
                xt_next = xp.tile([P, KO, TQ], BF16, tag="xt")
                for k in range(KO):
                    nc.sync.dma_start(xt_next[:, k],
                                      xTr[:, k, (ts + 1) * TQ:(ts + 2) * TQ])
            emit_attn(qs=ts)  # proj(ts-1) is interleaved into pair 0

        while pending_norm:
            emit_norm(pending_norm.pop(0))
        for t, pp in half_proj:  # finish the pair-1 half and drain
            for ns in range(C // TQ):
                mm(pp[:, ns * TQ:(ns + 1) * TQ],
                   Ot[:, 1, t * P:(t + 1) * P],
                   wp_sb[:, 1, ns * TQ:(ns + 1) * TQ],
                   start=False, stop=True, skip_group_check=True)
            ob = obp.tile([P, C], F32, tag="ob")
            nc.vector.tensor_add(ob[:], pp[:], bp_sb[:])
            nc.sync.dma_start(y[t * P:(t + 1) * P, :], ob[:])
        for t in range(4 * (NTS - 1) + 1, 4 * NTS):
            emit_proj(t, nc.vector)
        if dbg:
            qtf = consts.tile([P, 2, T], F32, tag="qtf")
            for g in range(2):
                nc.vector.tensor_copy(qtf[:, g], QT[:, g])
            nc.sync.dma_start(dbg["qt"].ap(), qtf[:])
            for g in range(2):
                nc.vector.tensor_copy(qtf[:, g], KTt[:, g])
            nc.sync.dma_start(dbg["kt"].ap(), qtf[:])
            vtf = consts.tile([P, NTK, HPC, DH + 1], F32, tag="vtf")
            nc.vector.tensor_copy(vtf[:], Vt[:])
            nc.sync.dma_start(dbg["vt"].ap(), vtf[:])
            for g in range(2):
                nc.vector.tensor_copy(qtf[:, g], Ot[:, g])
            nc.sync.dma_start(dbg["ot"].ap(), qtf[:])


def make_in_maps(x, W_attn, b_attn, W_proj, b_proj, dt=None):
    """Shard full inputs into per-core input maps."""
    import ml_dtypes
    bf16 = ml_dtypes.bfloat16
    x = np.asarray(x, np.float32)
    W_attn = np.asarray(W_attn, np.float32)
    b_attn = np.asarray(b_attn, np.float32)
    W_proj = np.asarray(W_proj, np.float32)
    b_proj = np.asarray(b_proj, np.float32)

    # [128, 132]: cols 0..127 = causal triangle (col >= row), col 128 = ones
    mask = (np.arange(132)[None, :] >= np.arange(P)[:, None]).astype(bf16)
    mask[:, 129] = 0
    in_maps = []
    for c in range(NCORES):
        b, hb = c // GROUP, c % GROUP
        cs = slice(hb * 256, (hb + 1) * 256)
        wq = W_attn[:, 0 * C:1 * C][:, cs]
        wk = W_attn[:, 1 * C:2 * C][:, cs]
        wv = W_attn[:, 2 * C:3 * C][:, cs]
        in_maps.append({
            "xT": np.ascontiguousarray(x[b].T).astype(bf16),
            "wqk": np.ascontiguousarray(np.concatenate([wq, wk], axis=1)).astype(bf16),
            "wv": np.ascontiguousarray(wv).astype(bf16),
            "wp": np.ascontiguousarray(W_proj[cs, :]),
            "bqk": np.concatenate([b_attn[0 * C:1 * C][cs], b_attn[1 * C:2 * C][cs]]),
            "bv": np.ascontiguousarray(b_attn[2 * C:3 * C][cs]),
            "bp": (b_proj if hb == 0 else np.zeros_like(b_proj)),
            "mask": mask,
        })
    return in_maps


def get_nc(dt=None, n_iters=1, **kw):
    key = ("nc", n_iters)
    if key not in _CACHE:
        _CACHE[key] = _build_nc(n_iters)
    return _CACHE[key]


def unshard(results):
    y = np.zeros((B, T, C), np.float32)
    for c in range(NCORES):
        y[c // GROUP] += results[c]["y"]
    return y


def kernel(x, W_attn, b_attn, W_proj, b_proj):
    nc = get_nc()
    in_maps = make_in_maps(x, W_attn, b_attn, W_proj, b_proj)
    res = run_bass_kernel_spmd(nc, in_maps, list(range(NCORES)))
    return unshard(res.results)
